# revision 1
# baseline (speedup 1.0000x reference)
"""Causal single-head attention (B=4, T=4096, D=1024, H=64) on 8 TRN2 cores.

Sharding: core c -> batch b=c//2, parity p=c%2. Each core computes attention
output for the 16 interleaved query tiles {128*(2i+p)} of its batch.  The
program is SPMD-uniform: per-core differences (which rows, causal masks) are
carried entirely in the input data (host-side slicing + mask tiles).

Device program per core:
  xT [1024,4096]  = x[b].T with own query columns first, partner's second
  qk-pass: [Wq|Wk] @ xT_own  -> qT [64,2048] (rows 0:64), kT own half (64:128)
  kv-pass: [Wk|Wv] @ xT_oth  -> kT other half, vT other half
  v-pass:   Wv    @ xT_own  -> vT own half
  vT -> v (natural [s,65] incl. ones column) via PE transposes
  per 512-row query span j: S^T tiles = kT_chunk.T @ qT_span (PSUM),
  exp on ACT (scale=1/8 folded in), causal masking = multiply with one of 8
  repeating mask tiles, PV matmul accumulates [v|1].T @ expS^T -> [65,512]
  (row 64 = sumexp), PE-transpose + reciprocal + tensor_scalar -> out.
"""

import os
import re
import numpy as np

B, T, D, H = 4, 4096, 1024, 64
NT = T // 128          # 32 key tiles per batch
NOWN = NT // 2         # 16 query tiles per core
F32 = None             # set lazily (mybir.dt.float32)

_PROG = None
LAST_EXEC_TIME_NS = None
LAST_RESULTS = None


def _patch_tile_drain():
    """Walrus in this container allows only one sync-wait on NO_STRUCT
    instructions; TileContext's tail drain carries one wait per DMA lane.
    Split it into one drain per outstanding proc."""
    import bass_rust
    import concourse.tile as tile

    if getattr(tile.TileContext, "_drain_patched", False):
        return

    def _drain_and_barrier(self, tick_clock, wait_clock):
        nc = self.nc
        gvec = tick_clock.global_clock
        ticks = eval(re.match(r"VectorClock\((\[.*\])\)", repr(gvec)).group(1))
        for pr, tk in enumerate(ticks):
            if tk > 0:
                vec = [0] * len(ticks)
                vec[pr] = tk
                d = nc.sync.drain()
                wait_clock.add_sem_waits(
                    d.ins,
                    bass_rust.ScopedClock({None: bass_rust.VectorClock(vec)}),
                )
        nc.sync.drain()
        nc.all_engine_barrier()
        assert self.sems is not None
        popped = nc._tile_sem_poison_stack.pop()
        assert popped is self._sem_poison
        nc.clear_and_free_semaphores(list(self.sems.allocated().values()))
        nc.all_engine_barrier()

    tile.TileContext._drain_and_barrier = _drain_and_barrier
    tile.TileContext._drain_patched = True


def _split_multi_waits(nc):
    """This walrus build allows at most one sync-wait per instruction.
    Hoist extra waits onto injected same-engine NOPs placed just before the
    owning instruction (same engine stream => identical semantics)."""
    import bass_rust

    for bb in nc.main_func.blocks:
        new_list = []
        for ins in bb.instructions:
            si = ins.sync_info
            if si is not None and si.on_wait and len(si.on_wait) > 1:
                waits = list(si.on_wait)
                for w in waits[:-1]:
                    nop = nc.engines[ins.engine].nop().ins
                    # remove the nop from wherever engine.nop() appended it
                    for bb2 in nc.main_func.blocks:
                        if nop in bb2.instructions:
                            bb2.instructions.remove(nop)
                            break
                    nop.sync_info = bass_rust.SyncInfo(on_wait=[w], on_update=[])
                    new_list.append(nop)
                si.on_wait = [waits[-1]]
            new_list.append(ins)
        bb.instructions[:] = new_list


def _build_program():
    import concourse.bass as bass
    import concourse.tile as tile
    from concourse import mybir
    from concourse.masks import make_identity

    _patch_tile_drain()
    f32 = mybir.dt.float32

    nc = bass.Bass()
    xT = nc.dram_tensor("xT", [D, T], f32, kind="ExternalInput")
    wqk = nc.dram_tensor("wqk", [D, 128], f32, kind="ExternalInput")
    wkv = nc.dram_tensor("wkv", [D, 128], f32, kind="ExternalInput")
    wv = nc.dram_tensor("wv", [D, H], f32, kind="ExternalInput")
    masks = nc.dram_tensor("masks", [8, 128, 512], f32, kind="ExternalInput")
    out = nc.dram_tensor("out", [T // 2, H], f32, kind="ExternalOutput")

    ND = D // 128  # 8 d-tiles

    with tile.TileContext(nc) as tc:
        with (
            tc.tile_pool(name="singles", bufs=1) as singles,
            tc.tile_pool(name="xt", bufs=3) as xtp,
            tc.tile_pool(name="pp", bufs=3) as ppool,
            tc.tile_pool(name="op", bufs=2) as opool,
        ):
            # ---- constant loads ----
            wqk_sb = singles.tile([128, ND, 128], f32)
            nc.sync.dma_start(out=wqk_sb, in_=wqk.rearrange("(dt p) h -> p dt h", p=128))
            wkv_sb = singles.tile([128, ND, 128], f32)
            nc.sync.dma_start(out=wkv_sb, in_=wkv.rearrange("(dt p) h -> p dt h", p=128))
            wv_sb = singles.tile([128, ND, H], f32)
            nc.sync.dma_start(out=wv_sb, in_=wv.rearrange("(dt p) h -> p dt h", p=128))
            mask_sb = singles.tile([128, 8, 512], f32)
            nc.sync.dma_start(out=mask_sb, in_=masks.rearrange("m p f -> p m f"))
            ident = singles.tile([128, 128], f32)
            make_identity(nc, ident)

            qT = singles.tile([64, T // 2], f32)          # q^T own rows
            kT = singles.tile([64, T], f32)               # k^T own-first layout
            vT_own = singles.tile([64, T // 2], f32)
            vT_oth = singles.tile([64, T // 2], f32)
            v_sb = singles.tile([128, NT, H + 1], f32)    # v natural + ones col
            out_sb = singles.tile([128, NOWN, H], f32)

            nc.vector.memset(v_sb[:, :, H : H + 1], 1.0)

            with tc.tile_pool(name="psA", bufs=1, space="PSUM") as psA:
                qk_ps = psA.tile([128, T // 2], f32, tag="qkkv")
                v_ps = psA.tile([64, T // 2], f32, tag="vps")
                # ---- pass A: [Wq|Wk] and Wv over own columns ----
                for d in range(ND):
                    xtd = xtp.tile([128, T // 2], f32, tag="xtd")
                    nc.sync.dma_start(out=xtd, in_=xT[d * 128 : (d + 1) * 128, 0 : T // 2])
                    for tck in range(4):
                        sl = slice(tck * 512, (tck + 1) * 512)
                        nc.tensor.matmul(qk_ps[:, sl], lhsT=wqk_sb[:, d, :], rhs=xtd[:, sl],
                                         start=(d == 0), stop=(d == ND - 1))
                        nc.tensor.matmul(v_ps[:, sl], lhsT=wv_sb[:, d, :], rhs=xtd[:, sl],
                                         start=(d == 0), stop=(d == ND - 1))
                nc.scalar.copy(out=qT, in_=qk_ps[0:64, :])
                nc.scalar.copy(out=kT[:, 0 : T // 2], in_=qk_ps[64:128, :])
                nc.vector.tensor_copy(out=vT_own, in_=v_ps[:, :])

                # ---- pass B: [Wk|Wv] over partner columns (reuses qkkv slot) ----
                kv_ps = psA.tile([128, T // 2], f32, tag="qkkv")
                for d in range(ND):
                    xtd = xtp.tile([128, T // 2], f32, tag="xtd")
                    nc.sync.dma_start(out=xtd, in_=xT[d * 128 : (d + 1) * 128, T // 2 : T])
                    for tck in range(4):
                        sl = slice(tck * 512, (tck + 1) * 512)
                        nc.tensor.matmul(kv_ps[:, sl], lhsT=wkv_sb[:, d, :], rhs=xtd[:, sl],
                                         start=(d == 0), stop=(d == ND - 1))
                nc.scalar.copy(out=kT[:, T // 2 : T], in_=kv_ps[0:64, :])
                nc.vector.tensor_copy(out=vT_oth, in_=kv_ps[64:128, :])

            # ---- attention ----
            with tc.tile_pool(name="psB", bufs=1, space="PSUM") as psB:
                # v^T -> v natural via PE transposes (own chunk i -> slot i,
                # partner chunk i -> slot 16+i; matches kT own-first layout)
                for i in range(NOWN):
                    tp = psB.tile([128, H], f32, tag="otp", bufs=2)
                    nc.tensor.transpose(tp, vT_own[:, i * 128 : (i + 1) * 128], ident[0:64, 0:64])
                    nc.vector.tensor_copy(out=v_sb[:, i, 0:H], in_=tp)
                for i in range(NOWN):
                    tp = psB.tile([128, H], f32, tag="otp", bufs=2)
                    nc.tensor.transpose(tp, vT_oth[:, i * 128 : (i + 1) * 128], ident[0:64, 0:64])
                    nc.vector.tensor_copy(out=v_sb[:, NOWN + i, 0:H], in_=tp)

                for j in range(4):
                    nch = 8 * j + 8  # uniform chunk count for this span
                    qsl = slice(j * 512, (j + 1) * 512)
                    op_ps = psB.tile([65, 512], f32, tag="oacc", bufs=2)
                    # chunk order: own 0..4j+3, then partner 0..4j+3
                    chunks = [(c, c, c - 4 * j) for c in range(4 * j + 4)] + [
                        (4 * j + 4 + c, NOWN + c, 4 + c - 4 * j if c >= 4 * j else -1)
                        for c in range(4 * j + 4)
                    ]
                    for half in range(nch // 2):
                        sc_ps = psB.tile([128, 1024], f32, tag="sc", bufs=2)
                        p_sb = ppool.tile([128, 1024], f32, tag="p")
                        for k in range(2):
                            seq, st, m = chunks[2 * half + k]
                            nc.tensor.matmul(
                                sc_ps[:, k * 512 : (k + 1) * 512],
                                lhsT=kT[:, st * 128 : (st + 1) * 128],
                                rhs=qT[:, qsl], start=True, stop=True)
                        nc.scalar.activation(out=p_sb, in_=sc_ps,
                                             func=mybir.ActivationFunctionType.Exp,
                                             scale=0.125)
                        for k in range(2):
                            seq, st, m = chunks[2 * half + k]
                            if 0 <= m < 4:       # own straddle -> OA_m = mask m
                                nc.vector.tensor_mul(
                                    out=p_sb[:, k * 512 : (k + 1) * 512],
                                    in0=p_sb[:, k * 512 : (k + 1) * 512],
                                    in1=mask_sb[:, m, :])
                            elif 4 <= m < 8:     # partner straddle -> OB_{m-4} = mask m
                                nc.vector.tensor_mul(
                                    out=p_sb[:, k * 512 : (k + 1) * 512],
                                    in0=p_sb[:, k * 512 : (k + 1) * 512],
                                    in1=mask_sb[:, m, :])
                            nc.tensor.matmul(
                                op_ps, lhsT=v_sb[:, st, :],
                                rhs=p_sb[:, k * 512 : (k + 1) * 512],
                                start=(2 * half + k == 0),
                                stop=(2 * half + k == nch - 1))
                    o_sb = opool.tile([65, 512], f32, tag="o")
                    nc.scalar.copy(out=o_sb, in_=op_ps)
                    for u in range(4):
                        tp = psB.tile([128, 65], f32, tag="otp", bufs=2)
                        nc.tensor.transpose(tp, o_sb[:, u * 128 : (u + 1) * 128], ident[0:65, 0:65])
                        r_sb = opool.tile([128, 1], f32, tag="r", bufs=2)
                        nc.vector.reciprocal(r_sb, tp[:, H : H + 1])
                        nc.vector.tensor_scalar_mul(
                            out=out_sb[:, 4 * j + u, :], in0=tp[:, 0:H], scalar1=r_sb)

            nc.sync.dma_start(out=out.rearrange("(c p) h -> p c h", p=128), in_=out_sb)
    _split_multi_waits(nc)
    return nc


def _host_inputs(x, Wk, Wq, Wv):
    """Build the 8 per-core input maps."""
    maps = []
    wqk = np.ascontiguousarray(np.concatenate([Wq, Wk], axis=1), np.float32)
    wkv = np.ascontiguousarray(np.concatenate([Wk, Wv], axis=1), np.float32)
    wv = np.ascontiguousarray(Wv, np.float32)
    s = np.arange(128)[:, None]
    t = np.arange(512)[None, :]
    tpos = (2 * (t // 128)) * 128 + (t % 128)
    for c in range(8):
        b, p = c // 2, c % 2
        own = [2 * i + p for i in range(NOWN)]
        oth = [2 * i + (1 - p) for i in range(NOWN)]
        own_rows = np.concatenate([np.arange(g * 128, (g + 1) * 128) for g in own])
        oth_rows = np.concatenate([np.arange(g * 128, (g + 1) * 128) for g in oth])
        xb = x[b]
        xTc = np.ascontiguousarray(
            np.concatenate([xb[own_rows].T, xb[oth_rows].T], axis=1), np.float32)
        mk = np.zeros((8, 128, 512), np.float32)
        for m in range(4):
            mk[m] = ((2 * m) * 128 + s <= tpos)            # OA_m (own straddle)
            mk[4 + m] = ((2 * m + 1 - 2 * p) * 128 + s <= tpos)  # OB_m (partner)
        maps.append({"xT": xTc, "wqk": wqk, "wkv": wkv, "wv": wv, "masks": mk})
    return maps


def kernel(x, Wk, Wq, Wv):
    global _PROG, LAST_EXEC_TIME_NS, LAST_RESULTS
    from concourse.bass_utils import run_bass_kernel_spmd

    if _PROG is None:
        _PROG = _build_program()
    in_maps = _host_inputs(np.asarray(x, np.float32), np.asarray(Wk, np.float32),
                           np.asarray(Wq, np.float32), np.asarray(Wv, np.float32))
    trace = os.environ.get("BASS_KERNEL_TRACE", "0") == "1"
    res = run_bass_kernel_spmd(_PROG, in_maps, list(range(8)), trace=trace)
    LAST_EXEC_TIME_NS = res.exec_time_ns
    LAST_RESULTS = res
    out = np.zeros((B, T, H), np.float32)
    for c in range(8):
        b, p = c // 2, c % 2
        oc = res.results[c]["out"]
        for i in range(NOWN):
            g = 2 * i + p
            out[b, g * 128 : (g + 1) * 128] = oc[i * 128 : (i + 1) * 128]
    return out



# revision 6
# speedup vs baseline: 2.1511x; 2.1511x over previous
"""Causal single-head attention (B=4, T=4096, D=1024, H=64) on 8 TRN2 cores.

Sharding: core c -> batch b=c//2, parity p=c%2. Each core computes attention
output for the 16 interleaved query tiles {128*(2i+p)} of its batch.  The
program is SPMD-uniform: per-core differences (which rows, causal masks) are
carried entirely in the input data (host-side slicing + mask tiles).

All matmul/elementwise inputs are bf16 (host-converted); accumulation stays
fp32 in PSUM and the output path (PV accumulate, sumexp, reciprocal, final
scale) stays fp32.

Device program per core:
  xT [1024,4096]  = x[b].T (bf16) with own query columns first, partner's last
  qk-pass: [Wq|Wk] @ xT_own  -> qT [64,2048] (rows 0:64), kT own half (64:128)
  kv-pass: [Wk|Wv] @ xT_oth  -> kT other half, vT other half
  v-pass:   Wv    @ xT_own  -> vT own half
  vT -> v (natural [s,65] incl. ones column) via PE transposes
  per 512-row query span j: S^T tiles = kT_chunk.T @ qT_span (PSUM),
  exp on ACT (scale=1/8 folded in) -> bf16, causal masking = multiply with one
  of 8 repeating mask tiles (DVE, bf16), PV matmul accumulates
  [v|1].T @ expS^T -> [65,512] fp32 (row 64 = sumexp), PE-transpose +
  reciprocal + tensor_scalar -> out.
"""

import os
import re
import numpy as np

B, T, D, H = 4, 4096, 1024, 64
NT = T // 128          # 32 key tiles per batch
NOWN = NT // 2         # 16 query tiles per core

_PROG = None
LAST_EXEC_TIME_NS = None
LAST_RESULTS = None


def _patch_tile_drain():
    """Walrus in this container allows only one sync-wait on NO_STRUCT
    instructions; TileContext's tail drain carries one wait per DMA lane.
    Split it into one drain per outstanding proc."""
    import bass_rust
    import concourse.tile as tile

    if getattr(tile.TileContext, "_drain_patched", False):
        return

    def _drain_and_barrier(self, tick_clock, wait_clock):
        nc = self.nc
        gvec = tick_clock.global_clock
        ticks = eval(re.match(r"VectorClock\((\[.*\])\)", repr(gvec)).group(1))
        for pr, tk in enumerate(ticks):
            if tk > 0:
                vec = [0] * len(ticks)
                vec[pr] = tk
                d = nc.sync.drain()
                wait_clock.add_sem_waits(
                    d.ins,
                    bass_rust.ScopedClock({None: bass_rust.VectorClock(vec)}),
                )
        nc.sync.drain()
        nc.all_engine_barrier()
        assert self.sems is not None
        popped = nc._tile_sem_poison_stack.pop()
        assert popped is self._sem_poison
        nc.clear_and_free_semaphores(list(self.sems.allocated().values()))
        nc.all_engine_barrier()

    tile.TileContext._drain_and_barrier = _drain_and_barrier
    tile.TileContext._drain_patched = True


def _split_multi_waits(nc):
    """This walrus build allows at most one sync-wait per instruction.
    Hoist extra waits onto injected same-engine NOPs placed just before the
    owning instruction (same engine stream => identical semantics)."""
    import bass_rust

    for bb in nc.main_func.blocks:
        new_list = []
        for ins in bb.instructions:
            si = ins.sync_info
            if si is not None and si.on_wait and len(si.on_wait) > 1:
                waits = list(si.on_wait)
                for w in waits[:-1]:
                    nop = nc.engines[ins.engine].nop().ins
                    # remove the nop from wherever engine.nop() appended it
                    for bb2 in nc.main_func.blocks:
                        if nop in bb2.instructions:
                            bb2.instructions.remove(nop)
                            break
                    nop.sync_info = bass_rust.SyncInfo(on_wait=[w], on_update=[])
                    new_list.append(nop)
                si.on_wait = [waits[-1]]
            new_list.append(ins)
        bb.instructions[:] = new_list


def _build_program():
    import concourse.bass as bass
    import concourse.tile as tile
    from concourse import mybir
    from concourse.masks import make_identity

    _patch_tile_drain()
    f32 = mybir.dt.float32
    bf16 = mybir.dt.bfloat16

    nc = bass.Bass()
    xT = nc.dram_tensor("xT", [D, T], bf16, kind="ExternalInput")
    wqk = nc.dram_tensor("wqk", [D, 128], bf16, kind="ExternalInput")
    wkv = nc.dram_tensor("wkv", [D, 128], bf16, kind="ExternalInput")
    wv = nc.dram_tensor("wv", [D, H], bf16, kind="ExternalInput")
    masks = nc.dram_tensor("masks", [8, 128, 512], bf16, kind="ExternalInput")
    out = nc.dram_tensor("out", [T // 2, H], f32, kind="ExternalOutput")

    ND = D // 128  # 8 d-tiles

    with tile.TileContext(nc) as tc:
        with (
            tc.tile_pool(name="singles", bufs=1) as singles,
            tc.tile_pool(name="xt", bufs=3) as xtp,
            tc.tile_pool(name="pp", bufs=3) as ppool,
            tc.tile_pool(name="op", bufs=2) as opool,
        ):
            # ---- constant loads ----
            wqk_sb = singles.tile([128, ND, 128], bf16)
            nc.sync.dma_start(out=wqk_sb, in_=wqk.rearrange("(dt p) h -> p dt h", p=128))
            wkv_sb = singles.tile([128, ND, 128], bf16)
            nc.sync.dma_start(out=wkv_sb, in_=wkv.rearrange("(dt p) h -> p dt h", p=128))
            wv_sb = singles.tile([128, ND, H], bf16)
            nc.sync.dma_start(out=wv_sb, in_=wv.rearrange("(dt p) h -> p dt h", p=128))
            mask_sb = singles.tile([128, 8, 512], bf16)
            nc.sync.dma_start(out=mask_sb, in_=masks.rearrange("m p f -> p m f"))
            ident_b = singles.tile([128, 128], bf16)
            make_identity(nc, ident_b)

            qT = singles.tile([64, T // 2], bf16)          # q^T own rows
            kT = singles.tile([64, T], bf16)               # k^T own-first layout
            vT_own = singles.tile([64, T // 2], bf16)
            vT_oth = singles.tile([64, T // 2], bf16)
            v_sb = singles.tile([128, NT, H + 1], bf16)    # v natural + ones col
            out_sb = singles.tile([128, NOWN, H], f32)

            nc.vector.memset(v_sb[:, :, H : H + 1], 1.0)

            with tc.tile_pool(name="psA", bufs=1, space="PSUM") as psA:
                qk_ps = psA.tile([128, T // 2], f32, tag="qkkv")
                v_ps = psA.tile([64, T // 2], f32, tag="vps")
                # ---- pass A: [Wq|Wk] and Wv over own columns ----
                for d in range(ND):
                    xtd = xtp.tile([128, T // 2], bf16, tag="xtd")
                    nc.sync.dma_start(out=xtd, in_=xT[d * 128 : (d + 1) * 128, 0 : T // 2])
                    for tck in range(4):
                        sl = slice(tck * 512, (tck + 1) * 512)
                        nc.tensor.matmul(qk_ps[:, sl], lhsT=wqk_sb[:, d, :], rhs=xtd[:, sl],
                                         start=(d == 0), stop=(d == ND - 1))
                        nc.tensor.matmul(v_ps[:, sl], lhsT=wv_sb[:, d, :], rhs=xtd[:, sl],
                                         start=(d == 0), stop=(d == ND - 1))
                nc.scalar.copy(out=qT, in_=qk_ps[0:64, :])
                nc.scalar.copy(out=kT[:, 0 : T // 2], in_=qk_ps[64:128, :])
                nc.vector.tensor_copy(out=vT_own, in_=v_ps[:, :])

                # ---- pass B: [Wk|Wv] over partner columns (reuses qkkv slot) ----
                kv_ps = psA.tile([128, T // 2], f32, tag="qkkv")
                for d in range(ND):
                    xtd = xtp.tile([128, T // 2], bf16, tag="xtd")
                    nc.sync.dma_start(out=xtd, in_=xT[d * 128 : (d + 1) * 128, T // 2 : T])
                    for tck in range(4):
                        sl = slice(tck * 512, (tck + 1) * 512)
                        nc.tensor.matmul(kv_ps[:, sl], lhsT=wkv_sb[:, d, :], rhs=xtd[:, sl],
                                         start=(d == 0), stop=(d == ND - 1))
                nc.scalar.copy(out=kT[:, T // 2 : T], in_=kv_ps[0:64, :])
                nc.vector.tensor_copy(out=vT_oth, in_=kv_ps[64:128, :])

            # ---- attention ----
            with tc.tile_pool(name="psB", bufs=1, space="PSUM") as psB:
                # v^T -> v natural via PE transposes (own chunk i -> slot i,
                # partner chunk i -> slot 16+i; matches kT own-first layout)
                for i in range(NOWN):
                    tp = psB.tile([128, 65], bf16, tag="otp", bufs=2)
                    nc.tensor.transpose(tp[:, 0:H], vT_own[:, i * 128 : (i + 1) * 128], ident_b[0:64, 0:64])
                    nc.vector.tensor_copy(out=v_sb[:, i, 0:H], in_=tp[:, 0:H])
                for i in range(NOWN):
                    tp = psB.tile([128, 65], bf16, tag="otp", bufs=2)
                    nc.tensor.transpose(tp[:, 0:H], vT_oth[:, i * 128 : (i + 1) * 128], ident_b[0:64, 0:64])
                    nc.vector.tensor_copy(out=v_sb[:, NOWN + i, 0:H], in_=tp[:, 0:H])

                for j in range(4):
                    nch = 8 * j + 8  # uniform chunk count for this span
                    qsl = slice(j * 512, (j + 1) * 512)
                    op_ps = psB.tile([65, 512], f32, tag="oacc", bufs=2)
                    # chunk order: own 0..4j+3, then partner 0..4j+3
                    chunks = [(c, c, c - 4 * j) for c in range(4 * j + 4)] + [
                        (4 * j + 4 + c, NOWN + c, 4 + c - 4 * j if c >= 4 * j else -1)
                        for c in range(4 * j + 4)
                    ]
                    for half in range(nch // 2):
                        sc_ps = psB.tile([128, 1024], f32, tag="sc", bufs=2)
                        p_sb = ppool.tile([128, 1024], bf16, tag="p")
                        for k in range(2):
                            seq, st, m = chunks[2 * half + k]
                            nc.tensor.matmul(
                                sc_ps[:, k * 512 : (k + 1) * 512],
                                lhsT=kT[:, st * 128 : (st + 1) * 128],
                                rhs=qT[:, qsl], start=True, stop=True)
                        nc.scalar.activation(out=p_sb, in_=sc_ps,
                                             func=mybir.ActivationFunctionType.Exp,
                                             scale=0.125)
                        for k in range(2):
                            seq, st, m = chunks[2 * half + k]
                            if 0 <= m < 8:       # straddle -> masked multiply
                                nc.vector.tensor_mul(
                                    out=p_sb[:, k * 512 : (k + 1) * 512],
                                    in0=p_sb[:, k * 512 : (k + 1) * 512],
                                    in1=mask_sb[:, m, :])
                            nc.tensor.matmul(
                                op_ps, lhsT=v_sb[:, st, :],
                                rhs=p_sb[:, k * 512 : (k + 1) * 512],
                                start=(2 * half + k == 0),
                                stop=(2 * half + k == nch - 1))
                    o_sb = opool.tile([65, 512], bf16, tag="o")
                    nc.vector.tensor_copy(out=o_sb, in_=op_ps)
                    for u in range(4):
                        tp = psB.tile([128, 65], bf16, tag="otp", bufs=2)
                        nc.tensor.transpose(tp, o_sb[:, u * 128 : (u + 1) * 128], ident_b[0:65, 0:65])
                        r_sb = opool.tile([128, 1], f32, tag="r", bufs=2)
                        nc.vector.reciprocal(r_sb, tp[:, H : H + 1])
                        nc.vector.tensor_scalar_mul(
                            out=out_sb[:, 4 * j + u, :], in0=tp[:, 0:H], scalar1=r_sb)

            nc.sync.dma_start(out=out.rearrange("(c p) h -> p c h", p=128), in_=out_sb)
    _split_multi_waits(nc)
    return nc


def _host_inputs(x, Wk, Wq, Wv):
    """Build the 8 per-core input maps (all compute tensors bf16)."""
    import ml_dtypes

    bf = ml_dtypes.bfloat16
    maps = []
    wqk = np.concatenate([Wq, Wk], axis=1).astype(bf)
    wkv = np.concatenate([Wk, Wv], axis=1).astype(bf)
    wv = np.ascontiguousarray(Wv).astype(bf)
    s = np.arange(128)[:, None]
    t = np.arange(512)[None, :]
    tpos = (2 * (t // 128)) * 128 + (t % 128)
    xbf = x.astype(bf)
    for c in range(8):
        b, p = c // 2, c % 2
        own = [2 * i + p for i in range(NOWN)]
        oth = [2 * i + (1 - p) for i in range(NOWN)]
        own_rows = np.concatenate([np.arange(g * 128, (g + 1) * 128) for g in own])
        oth_rows = np.concatenate([np.arange(g * 128, (g + 1) * 128) for g in oth])
        xb = xbf[b]
        xTc = np.ascontiguousarray(
            np.concatenate([xb[own_rows].T, xb[oth_rows].T], axis=1))
        mk = np.zeros((8, 128, 512), np.float32)
        for m in range(4):
            mk[m] = ((2 * m) * 128 + s <= tpos)            # OA_m (own straddle)
            mk[4 + m] = ((2 * m + 1 - 2 * p) * 128 + s <= tpos)  # OB_m (partner)
        maps.append({"xT": xTc, "wqk": wqk, "wkv": wkv, "wv": wv,
                     "masks": mk.astype(bf)})
    return maps


def kernel(x, Wk, Wq, Wv):
    global _PROG, LAST_EXEC_TIME_NS, LAST_RESULTS
    from concourse.bass_utils import run_bass_kernel_spmd

    if _PROG is None:
        _PROG = _build_program()
    in_maps = _host_inputs(np.asarray(x, np.float32), np.asarray(Wk, np.float32),
                           np.asarray(Wq, np.float32), np.asarray(Wv, np.float32))
    trace = os.environ.get("BASS_KERNEL_TRACE", "0") == "1"
    res = run_bass_kernel_spmd(_PROG, in_maps, list(range(8)), trace=trace)
    LAST_EXEC_TIME_NS = res.exec_time_ns
    LAST_RESULTS = res
    out = np.zeros((B, T, H), np.float32)
    for c in range(8):
        b, p = c // 2, c % 2
        oc = res.results[c]["out"]
        for i in range(NOWN):
            g = 2 * i + p
            out[b, g * 128 : (g + 1) * 128] = oc[i * 128 : (i + 1) * 128]
    return out


# revision 8
# speedup vs baseline: 2.4771x; 1.1515x over previous
"""Causal single-head attention (B=4, T=4096, D=1024, H=64) on 8 TRN2 cores.

Sharding: core c -> batch b=c//2, parity p=c%2. Each core computes attention
output for the 16 interleaved query tiles {128*(2i+p)} of its batch.  The
program is SPMD-uniform: per-core differences (which rows, causal masks) are
carried entirely in the input data (host-side slicing + mask tiles).

All matmul/elementwise inputs are bf16; accumulation is fp32 in PSUM.

Key-tile coverage is exact-causal: for query-span j (own q-tiles 4j..4j+3,
512 cols), own-key chunks are slots 0..4j-1 full-width plus the 4 "diagonal"
slots 4j+u covering span cols [128u:512) with a triangular mask on the first
128 cols; partner-key chunks mirror this with an all-ones/all-zeros mask (OB)
capturing the parity difference.  Score columns are packed into [128,1024]
PSUM groups so one ACT exp instruction covers ~2 chunks.

Order of emission: passA (own x cols -> q,k_own,v_own) -> own-key attention
(all spans; PV partials parked in SBUF) -> passB (partner x cols -> k_oth,
v_oth; its DMA streams during own attention) -> partner-key attention +
combine + epilogue.
"""

import os
import re
import numpy as np

B, T, D, H = 4, 4096, 1024, 64
NT = T // 128          # 32 key tiles per batch
NOWN = NT // 2         # 16 query tiles per core
GW = 1024              # score-group width (cols) = one ACT exp instruction

_PROG = None
LAST_EXEC_TIME_NS = None
LAST_RESULTS = None


def _patch_tile_drain():
    """Walrus in this container allows only one sync-wait on NO_STRUCT
    instructions; TileContext's tail drain carries one wait per DMA lane.
    Split it into one drain per outstanding proc."""
    import bass_rust
    import concourse.tile as tile

    if getattr(tile.TileContext, "_drain_patched", False):
        return

    def _drain_and_barrier(self, tick_clock, wait_clock):
        nc = self.nc
        gvec = tick_clock.global_clock
        ticks = eval(re.match(r"VectorClock\((\[.*\])\)", repr(gvec)).group(1))
        for pr, tk in enumerate(ticks):
            if tk > 0:
                vec = [0] * len(ticks)
                vec[pr] = tk
                d = nc.sync.drain()
                wait_clock.add_sem_waits(
                    d.ins,
                    bass_rust.ScopedClock({None: bass_rust.VectorClock(vec)}),
                )
        nc.sync.drain()
        nc.all_engine_barrier()
        assert self.sems is not None
        popped = nc._tile_sem_poison_stack.pop()
        assert popped is self._sem_poison
        nc.clear_and_free_semaphores(list(self.sems.allocated().values()))
        nc.all_engine_barrier()

    tile.TileContext._drain_and_barrier = _drain_and_barrier
    tile.TileContext._drain_patched = True


def _split_multi_waits(nc):
    """This walrus build allows at most one sync-wait per instruction.
    Hoist extra waits onto injected same-engine NOPs placed just before the
    owning instruction (same engine stream => identical semantics)."""
    import bass_rust

    for bb in nc.main_func.blocks:
        new_list = []
        for ins in bb.instructions:
            si = ins.sync_info
            if si is not None and si.on_wait and len(si.on_wait) > 1:
                waits = list(si.on_wait)
                for w in waits[:-1]:
                    nop = nc.engines[ins.engine].nop().ins
                    # remove the nop from wherever engine.nop() appended it
                    for bb2 in nc.main_func.blocks:
                        if nop in bb2.instructions:
                            bb2.instructions.remove(nop)
                            break
                    nop.sync_info = bass_rust.SyncInfo(on_wait=[w], on_update=[])
                    new_list.append(nop)
                si.on_wait = [waits[-1]]
            new_list.append(ins)
        bb.instructions[:] = new_list


def _span_chunks(j, partner):
    """Chunk list for span j: (slot, c0, w, mask_idx|None). Span cols are
    [j*512, (j+1)*512); c0/w are within-span. mask applies to chunk cols
    [c0, c0+128)."""
    base = 16 if partner else 0
    out = []
    for wslot in range(4 * j):
        out.append((base + wslot, 0, 512, None))
    mi = 1 if partner else 0  # 0=TRI (own diagonal), 1=OB (parity gate)
    for u in range(4):
        out.append((base + 4 * j + u, 128 * u, 512 - 128 * u, mi))
    return out


def _pack_groups(chunks):
    """Greedy-pack chunk columns into groups of <=GW cols (splitting only at
    128-col boundaries, never inside the first 128 cols of a masked chunk).
    Returns list of groups; each group is (width, segs) with
    segs = [(chunk_idx, chunk_off, group_off, w)]."""
    groups = []
    cur = []
    cur_w = 0
    for ci, (slot, c0, w, mi) in enumerate(chunks):
        off = 0
        while off < w:
            room = GW - cur_w
            if room == 0:
                groups.append((cur_w, cur))
                cur, cur_w = [], 0
                room = GW
            take = min(room, w - off)
            if mi is not None and off == 0 and take < 128:
                # don't sever the masked head block; start a fresh group
                groups.append((cur_w, cur))
                cur, cur_w = [], 0
                take = min(GW, w)
            cur.append((ci, off, cur_w, take))
            cur_w += take
            off += take
    if cur:
        groups.append((cur_w, cur))
    return groups


def _build_program():
    import concourse.bass as bass
    import concourse.tile as tile
    from concourse import mybir
    from concourse.masks import make_identity

    _patch_tile_drain()
    f32 = mybir.dt.float32
    bf16 = mybir.dt.bfloat16

    nc = bass.Bass()
    xT = nc.dram_tensor("xT", [D, T], bf16, kind="ExternalInput")
    wqk = nc.dram_tensor("wqk", [128, D], bf16, kind="ExternalInput")
    wkv = nc.dram_tensor("wkv", [128, D], bf16, kind="ExternalInput")
    wv = nc.dram_tensor("wv", [128, D // 128 * H], bf16, kind="ExternalInput")
    masks = nc.dram_tensor("masks", [128, 2 * 128], bf16, kind="ExternalInput")
    out = nc.dram_tensor("out", [128, NOWN * H], f32, kind="ExternalOutput")

    ND = D // 128  # 8 d-tiles

    with tile.TileContext(nc) as tc:
        with (
            tc.tile_pool(name="singles", bufs=1) as singles,
            tc.tile_pool(name="xta", bufs=8) as xta,
            tc.tile_pool(name="xtb", bufs=8) as xtb,
            tc.tile_pool(name="pp", bufs=3) as ppool,
            tc.tile_pool(name="op", bufs=2) as opool,
        ):
            # ---- constant loads (host supplies partition-major layouts) ----
            wqk_sb = singles.tile([128, ND, 128], bf16)
            nc.sync.dma_start(out=wqk_sb, in_=wqk.rearrange("p (dt h) -> p dt h", dt=ND))
            wkv_sb = singles.tile([128, ND, 128], bf16)
            nc.sync.dma_start(out=wkv_sb, in_=wkv.rearrange("p (dt h) -> p dt h", dt=ND))
            wv_sb = singles.tile([128, ND, H], bf16)
            nc.sync.dma_start(out=wv_sb, in_=wv.rearrange("p (dt h) -> p dt h", dt=ND))
            mask_sb = singles.tile([128, 2, 128], bf16)
            nc.sync.dma_start(out=mask_sb, in_=masks.rearrange("p (m f) -> p m f", m=2))
            ident_b = singles.tile([128, 128], bf16)
            make_identity(nc, ident_b)

            # x tiles: own columns first, partner columns second
            xa = []
            for d in range(ND):
                t_ = xta.tile([128, T // 2], bf16, tag="xa")
                nc.sync.dma_start(out=t_, in_=xT[d * 128 : (d + 1) * 128, 0 : T // 2])
                xa.append(t_)
            xb = []
            for d in range(ND):
                t_ = xtb.tile([128, T // 2], bf16, tag="xb")
                nc.sync.dma_start(out=t_, in_=xT[d * 128 : (d + 1) * 128, T // 2 : T])
                xb.append(t_)

            qT = singles.tile([64, T // 2], bf16)          # q^T own rows
            kT = singles.tile([64, T], bf16)               # k^T own-first layout
            vT_own = singles.tile([64, T // 2], bf16)
            vT_oth = singles.tile([64, T // 2], bf16)
            v_sb = singles.tile([128, NT, H + 1], bf16)    # v natural + ones col
            o_own = singles.tile([65, 4, 512], f32)        # own-key PV partials
            out_sb = singles.tile([128, NOWN, H], f32)

            nc.vector.memset(v_sb[:, :, H : H + 1], 1.0)

            # ---- pass A: [Wq|Wk] and Wv over own columns ----
            with tc.tile_pool(name="psA", bufs=1, space="PSUM") as psA:
                qk_ps = psA.tile([128, T // 2], f32, tag="qk")
                v_ps = psA.tile([64, T // 2], f32, tag="vp")
                for d in range(ND):
                    for tck in range(4):
                        sl = slice(tck * 512, (tck + 1) * 512)
                        nc.tensor.matmul(qk_ps[:, sl], lhsT=wqk_sb[:, d, :], rhs=xa[d][:, sl],
                                         start=(d == 0), stop=(d == ND - 1))
                        nc.tensor.matmul(v_ps[:, sl], lhsT=wv_sb[:, d, :], rhs=xa[d][:, sl],
                                         start=(d == 0), stop=(d == ND - 1))
                nc.vector.tensor_copy(out=kT[:, 0 : T // 2], in_=qk_ps[64:128, :])
                nc.vector.tensor_copy(out=qT, in_=qk_ps[0:64, :])
                nc.vector.tensor_copy(out=vT_own, in_=v_ps[:, :])

            # ---- attention + passB ----
            with tc.tile_pool(name="psB", bufs=1, space="PSUM") as psB:
                # v_own^T -> natural layout (slots 0..15)
                for i in range(NOWN):
                    tp = psB.tile([128, 65], bf16, tag="otp", bufs=1)
                    nc.tensor.transpose(tp[:, 0:H], vT_own[:, i * 128 : (i + 1) * 128],
                                        ident_b[0:64, 0:64])
                    nc.vector.tensor_copy(out=v_sb[:, i, 0:H], in_=tp[:, 0:H])

                def emit_phase(partner, epilogue):
                    # build per-span groups, flatten into one pipelined stream
                    stream = []  # (j, width, segs, chunks, first_g_of_span, last_g_of_span)
                    for j in range(4):
                        chunks = _span_chunks(j, partner)
                        groups = _pack_groups(chunks)
                        for gi, (gwidth, segs) in enumerate(groups):
                            stream.append((j, gwidth, segs, chunks,
                                           gi == 0, gi == len(groups) - 1))

                    sc_tiles = [None] * len(stream)
                    p_tiles = [None] * len(stream)
                    op_tiles = {}

                    def emit_st(g):
                        j, gwidth, segs, chunks, first, last = stream[g]
                        sc = psB.tile([128, GW], f32, tag="sc", bufs=2)
                        sc_tiles[g] = sc
                        for (ci, coff, goff, w) in segs:
                            slot, c0, cw, mi = chunks[ci]
                            nc.tensor.matmul(
                                sc[:, goff : goff + w],
                                lhsT=kT[:, slot * 128 : (slot + 1) * 128],
                                rhs=qT[:, j * 512 + c0 + coff : j * 512 + c0 + coff + w],
                                start=True, stop=True)

                    def emit_exp(g):
                        j, gwidth, segs, chunks, first, last = stream[g]
                        p = ppool.tile([128, GW], bf16, tag="p")
                        p_tiles[g] = p
                        nc.scalar.activation(out=p[:, 0:gwidth], in_=sc_tiles[g][:, 0:gwidth],
                                             func=mybir.ActivationFunctionType.Exp,
                                             scale=0.125)

                    def emit_mask_pv(g):
                        j, gwidth, segs, chunks, first, last = stream[g]
                        p = p_tiles[g]
                        if first:
                            op_tiles[j] = opool_ps.tile([65, 512], f32, tag="oacc",
                                                        bufs=2, name=f"op_j{j}")
                        op = op_tiles[j]
                        for (ci, coff, goff, w) in segs:
                            slot, c0, cw, mi = chunks[ci]
                            if mi is not None and coff == 0:
                                nc.vector.tensor_mul(
                                    out=p[:, goff : goff + 128],
                                    in0=p[:, goff : goff + 128],
                                    in1=mask_sb[:, mi, :])
                        n_ch = len(chunks)
                        for (ci, coff, goff, w) in segs:
                            slot, c0, cw, mi = chunks[ci]
                            nc.tensor.matmul(
                                op[:, c0 + coff : c0 + coff + w],
                                lhsT=v_sb[:, slot, :],
                                rhs=p[:, goff : goff + w],
                                start=(ci == 0),
                                stop=(ci == n_ch - 1 and coff + w == cw))
                        if last:
                            if not partner:
                                nc.vector.tensor_copy(out=o_own[:, j, :], in_=op)
                            else:
                                epilogue(j, op)

                    # lag-1 software pipeline: ST(g) | exp(g-1)->... PV(g-1)
                    n = len(stream)
                    emit_st(0)
                    emit_exp(0)
                    for g in range(n):
                        if g + 1 < n:
                            emit_st(g + 1)
                            emit_exp(g + 1)
                        emit_mask_pv(g)

                opool_ps = psB  # alias; op tiles carved from psB pool

                emit_phase(False, None)

                # ---- pass B (chunk-major, 1-bank accumulator) ----
                with tc.tile_pool(name="psKV", bufs=1, space="PSUM") as psKV:
                    for tck in range(4):
                        kv = psKV.tile([128, 512], f32, tag="kv", bufs=1)
                        for d in range(ND):
                            nc.tensor.matmul(kv, lhsT=wkv_sb[:, d, :],
                                             rhs=xb[d][:, tck * 512 : (tck + 1) * 512],
                                             start=(d == 0), stop=(d == ND - 1))
                        sl = slice(tck * 512, (tck + 1) * 512)
                        nc.vector.tensor_copy(out=kT[:, T // 2 + tck * 512 : T // 2 + (tck + 1) * 512],
                                              in_=kv[0:64, :])
                        nc.vector.tensor_copy(out=vT_oth[:, sl], in_=kv[64:128, :])

                # v_oth^T -> natural layout (slots 16..31)
                for i in range(NOWN):
                    tp = psB.tile([128, 65], bf16, tag="otp", bufs=1)
                    nc.tensor.transpose(tp[:, 0:H], vT_oth[:, i * 128 : (i + 1) * 128],
                                        ident_b[0:64, 0:64])
                    nc.vector.tensor_copy(out=v_sb[:, NOWN + i, 0:H], in_=tp[:, 0:H])

                def epilogue(j, op):
                    o_sb = opool.tile([65, 512], bf16, tag="o")
                    nc.vector.tensor_add(out=o_sb, in0=op, in1=o_own[:, j, :])
                    for u in range(4):
                        tp = psB.tile([128, 65], bf16, tag="otp", bufs=1)
                        nc.tensor.transpose(tp, o_sb[:, u * 128 : (u + 1) * 128],
                                            ident_b[0:65, 0:65])
                        r_sb = opool.tile([128, 1], f32, tag="r", bufs=2)
                        nc.vector.reciprocal(r_sb, tp[:, H : H + 1])
                        nc.vector.tensor_scalar_mul(
                            out=out_sb[:, 4 * j + u, :], in0=tp[:, 0:H], scalar1=r_sb)

                emit_phase(True, epilogue)

            nc.sync.dma_start(out=out.rearrange("p (c h) -> p c h", c=NOWN), in_=out_sb)
    _split_multi_waits(nc)
    return nc


def _host_inputs(x, Wk, Wq, Wv):
    """Build the 8 per-core input maps (partition-major constant layouts)."""
    import ml_dtypes

    bf = ml_dtypes.bfloat16
    ND = D // 128
    maps = []
    # [D, 128] -> [128p, ND, 128] -> [128, ND*128]
    wqk = np.concatenate([Wq, Wk], axis=1).reshape(ND, 128, 128).transpose(1, 0, 2)
    wqk = np.ascontiguousarray(wqk.reshape(128, ND * 128)).astype(bf)
    wkv = np.concatenate([Wk, Wv], axis=1).reshape(ND, 128, 128).transpose(1, 0, 2)
    wkv = np.ascontiguousarray(wkv.reshape(128, ND * 128)).astype(bf)
    wvm = Wv.reshape(ND, 128, H).transpose(1, 0, 2)
    wvm = np.ascontiguousarray(wvm.reshape(128, ND * H)).astype(bf)
    s = np.arange(128)[:, None]
    t = np.arange(128)[None, :]
    tri = (s <= t).astype(np.float32)  # diagonal triangle: key s, query col t
    xbf = x.astype(bf)
    for c in range(8):
        b, p = c // 2, c % 2
        own = [2 * i + p for i in range(NOWN)]
        oth = [2 * i + (1 - p) for i in range(NOWN)]
        own_rows = np.concatenate([np.arange(g * 128, (g + 1) * 128) for g in own])
        oth_rows = np.concatenate([np.arange(g * 128, (g + 1) * 128) for g in oth])
        xb_ = xbf[b]
        xTc = np.ascontiguousarray(
            np.concatenate([xb_[own_rows].T, xb_[oth_rows].T], axis=1))
        ob = np.full((128, 128), float(p), np.float32)  # partner straddle gate
        mk = np.stack([tri, ob], axis=1).reshape(128, 2 * 128)
        maps.append({"xT": xTc, "wqk": wqk, "wkv": wkv, "wv": wvm,
                     "masks": mk.astype(bf)})
    return maps


def kernel(x, Wk, Wq, Wv):
    global _PROG, LAST_EXEC_TIME_NS, LAST_RESULTS
    from concourse.bass_utils import run_bass_kernel_spmd

    if _PROG is None:
        _PROG = _build_program()
    in_maps = _host_inputs(np.asarray(x, np.float32), np.asarray(Wk, np.float32),
                           np.asarray(Wq, np.float32), np.asarray(Wv, np.float32))
    trace = os.environ.get("BASS_KERNEL_TRACE", "0") == "1"
    res = run_bass_kernel_spmd(_PROG, in_maps, list(range(8)), trace=trace)
    LAST_EXEC_TIME_NS = res.exec_time_ns
    LAST_RESULTS = res
    out = np.zeros((B, T, H), np.float32)
    for c in range(8):
        b, p = c // 2, c % 2
        oc = res.results[c]["out"].reshape(128, NOWN, H)
        for i in range(NOWN):
            g = 2 * i + p
            out[b, g * 128 : (g + 1) * 128] = oc[:, i, :]
    return out
